# revision 3
# baseline (speedup 1.0000x reference)
"""Trainium2 Bass kernel for AttributeAttentionModule.

y = attention over heads of QKV projections:
  Q = sa @ Wq.T + bq ; K = x @ Wk.T + bk ; V = x @ Wv.T + bv   (all [B, D])
  per-sample scores[h,g] = Q_h . K_g / 32 ; softmax over g ; out_h = sum_g w_hg V_g

Data-parallel over 8 NeuronCores (batch sharded). Matmuls run in float32r
(FP22) at 1 cycle/row. Weights are streamed once per group of 8 batch-tiles
(all 8 PSUM banks accumulate in parallel over the contraction dim).
"""

import os
import sys

for _p in ("/opt/trn_rl_repo", "/root/.axon_site/_ro/trn_rl_repo"):
    if os.path.isdir(_p) and _p not in sys.path:
        sys.path.append(_p)

import numpy as np
from contextlib import ExitStack

B = 16384
D = 3072
H = 3
DH = D // H          # 1024
NCORES = 8
P = 128              # partition tile
NO = 512             # matmul moving free dim (one PSUM bank of fp32)
KGRP = 6             # k-tiles per weight DMA

_CACHE = {}


def _build(bs=B // NCORES, gbt=8):
    """Build + compile the per-core program. bs = batch rows per core,
    gbt = batch tiles (of 128) per weight-streaming group."""
    import concourse.bass as bass
    import concourse.tile as tile
    from concourse import bacc, mybir

    f32 = mybir.dt.float32
    f32r = mybir.dt.float32r
    mult = mybir.AluOpType.mult
    add = mybir.AluOpType.add
    Exp = mybir.ActivationFunctionType.Exp

    nbt = bs // P        # batch tiles per core
    ng = nbt // gbt      # weight-stream groups
    kt = D // P          # 24 contraction tiles
    no = D // NO         # 6 output-column tiles
    nkg = kt // KGRP     # 4 weight DMAs per o-column

    nc = bacc.Bacc(
        "TRN2", target_bir_lowering=False, debug=False, num_devices=NCORES
    )

    saT = nc.dram_tensor("saT", [D, bs], f32r, kind="ExternalInput").ap()
    xT = nc.dram_tensor("xT", [D, bs], f32r, kind="ExternalInput").ap()
    wT = {
        t: nc.dram_tensor(f"w{t}T", [D, D], f32r, kind="ExternalInput").ap()
        for t in "qkv"
    }
    biasd = {
        t: nc.dram_tensor(f"b{t}", [P, D], f32, kind="ExternalInput").ap()
        for t in "qkv"
    }
    outd = nc.dram_tensor("out", [bs, D], f32, kind="ExternalOutput").ap()

    with tile.TileContext(nc) as tc, ExitStack() as ctx:
        dram = ctx.enter_context(tc.tile_pool(name="dram", bufs=1, space="DRAM"))
        qkv_s = {t: dram.tile([bs, D], f32, tag=f"s{t}", name=f"s{t}") for t in "qkv"}

        with ExitStack() as mm:
            apool = mm.enter_context(tc.tile_pool(name="apool", bufs=1))
            wpool = mm.enter_context(tc.tile_pool(name="wpool", bufs=3))
            bpool = mm.enter_context(tc.tile_pool(name="bpool", bufs=2))
            ocpool = mm.enter_context(tc.tile_pool(name="ocpool", bufs=4))
            pspool = mm.enter_context(
                tc.tile_pool(name="psum", bufs=1, space="PSUM")
            )

            def load_act(src, g):
                tiles = []
                for i in range(gbt):
                    t = apool.tile([P, kt, P], f32r, tag=f"a{i}", name=f"a{i}")
                    c0 = (g * gbt + i) * P
                    nc.sync.dma_start(
                        t[:],
                        src[:, c0 : c0 + P].rearrange("(ko p) b -> p ko b", p=P),
                    )
                    tiles.append(t)
                return tiles

            def proj(a_t, wTd, bias_d, dst, g):
                bias_t = bpool.tile([P, D], f32, tag="bias", name="bias")
                nc.sync.dma_start(bias_t[:], bias_d[:])
                for o in range(no):
                    ps = [
                        pspool.tile([P, NO], f32, tag=f"ps{i}", name=f"ps{i}") for i in range(gbt)
                    ]
                    for kg in range(nkg):
                        wt = wpool.tile([P, KGRP, NO], f32r, tag="w", name="w")
                        r0 = kg * KGRP * P
                        nc.sync.dma_start(
                            wt[:],
                            wTd[
                                r0 : r0 + KGRP * P, o * NO : (o + 1) * NO
                            ].rearrange("(ko p) n -> p ko n", p=P),
                        )
                        for j in range(KGRP):
                            k = kg * KGRP + j
                            for i in range(gbt):
                                nc.tensor.matmul(
                                    ps[i][:],
                                    a_t[i][:, k, :],
                                    wt[:, j, :],
                                    start=(k == 0),
                                    stop=(k == kt - 1),
                                )
                    for i in range(gbt):
                        oc = ocpool.tile([P, NO], f32, tag="oc", name="oc")
                        nc.vector.tensor_add(
                            oc[:], ps[i][:], bias_t[:, o * NO : (o + 1) * NO]
                        )
                        row0 = (g * gbt + i) * P
                        nc.sync.dma_start(
                            dst[row0 : row0 + P, o * NO : (o + 1) * NO], oc[:]
                        )

            for g in range(ng):
                sa_t = load_act(saT, g)
                proj(sa_t, wT["q"], biasd["q"], qkv_s["q"], g)
                x_t = load_act(xT, g)
                proj(x_t, wT["k"], biasd["k"], qkv_s["k"], g)
                proj(x_t, wT["v"], biasd["v"], qkv_s["v"], g)

        with ExitStack() as at:
            qkvp = at.enter_context(tc.tile_pool(name="qkvp", bufs=2))
            smallp = at.enter_context(tc.tile_pool(name="smallp", bufs=4))
            accp = at.enter_context(tc.tile_pool(name="accp", bufs=2))
            outp = at.enter_context(tc.tile_pool(name="outp", bufs=2))

            for bt in range(nbt):
                r0 = bt * P
                t3 = {}
                for t in "qkv":
                    tt = qkvp.tile([P, D], f32, tag=t, name=f"t_{t}")
                    nc.sync.dma_start(tt[:], qkv_s[t][r0 : r0 + P, :])
                    t3[t] = tt
                s = smallp.tile([P, H * H], f32, tag="s", name="s")
                prod = accp.tile([P, DH], f32, tag="prod", name="prod")
                for h in range(H):
                    for g2 in range(H):
                        # tensor_tensor_reduce would fuse these but crashes
                        # the exec unit on this runtime; use two ops.
                        nc.vector.tensor_mul(
                            prod[:],
                            t3["q"][:, h * DH : (h + 1) * DH],
                            t3["k"][:, g2 * DH : (g2 + 1) * DH],
                        )
                        nc.vector.reduce_sum(
                            s[:, h * H + g2 : h * H + g2 + 1],
                            prod[:],
                            axis=mybir.AxisListType.X,
                        )
                e = smallp.tile([P, H * H], f32, tag="e", name="e")
                nc.scalar.activation(e[:], s[:], Exp, scale=1.0 / 32.0)
                ssum = smallp.tile([P, H], f32, tag="ssum", name="ssum")
                nc.vector.tensor_reduce(
                    ssum[:],
                    e[:].rearrange("p (h g) -> p h g", h=H),
                    axis=mybir.AxisListType.X,
                    op=add,
                )
                rcp = smallp.tile([P, H], f32, tag="rcp", name="rcp")
                nc.vector.reciprocal(rcp[:], ssum[:])
                ot = outp.tile([P, D], f32, tag="o", name="o")
                for h in range(H):
                    acc = accp.tile([P, DH], f32, tag="acc", name="acc")
                    nc.vector.tensor_scalar_mul(
                        acc[:], t3["v"][:, 0:DH], e[:, h * H : h * H + 1]
                    )
                    for g2 in (1, 2):
                        nc.vector.scalar_tensor_tensor(
                            acc[:],
                            t3["v"][:, g2 * DH : (g2 + 1) * DH],
                            e[:, h * H + g2 : h * H + g2 + 1],
                            acc[:],
                            op0=mult,
                            op1=add,
                        )
                    nc.vector.tensor_scalar_mul(
                        ot[:, h * DH : (h + 1) * DH], acc[:], rcp[:, h : h + 1]
                    )
                nc.sync.dma_start(outd[r0 : r0 + P, :], ot[:])

    nc.compile()
    return nc


def _get_nc(bs=B // NCORES, gbt=8):
    key = (bs, gbt)
    if key not in _CACHE:
        _CACHE[key] = _build(bs, gbt)
    return _CACHE[key]


def kernel(x, synthetic_attributes, Wq, bq, Wk, bk, Wv, bv, **_ignored):
    from concourse import bass_utils

    x = np.asarray(x, dtype=np.float32)
    sa = np.asarray(synthetic_attributes, dtype=np.float32)
    bs = x.shape[0] // NCORES

    wqT = np.ascontiguousarray(np.asarray(Wq, dtype=np.float32).T)
    wkT = np.ascontiguousarray(np.asarray(Wk, dtype=np.float32).T)
    wvT = np.ascontiguousarray(np.asarray(Wv, dtype=np.float32).T)
    bqb = np.ascontiguousarray(
        np.broadcast_to(np.asarray(bq, dtype=np.float32), (P, D))
    )
    bkb = np.ascontiguousarray(
        np.broadcast_to(np.asarray(bk, dtype=np.float32), (P, D))
    )
    bvb = np.ascontiguousarray(
        np.broadcast_to(np.asarray(bv, dtype=np.float32), (P, D))
    )

    nc = _get_nc(bs=bs)

    in_maps = []
    for c in range(NCORES):
        r0 = c * bs
        in_maps.append(
            {
                "saT": np.ascontiguousarray(sa[r0 : r0 + bs].T),
                "xT": np.ascontiguousarray(x[r0 : r0 + bs].T),
                "wqT": wqT,
                "wkT": wkT,
                "wvT": wvT,
                "bq": bqb,
                "bk": bkb,
                "bv": bvb,
            }
        )

    res = bass_utils.run_bass_kernel_spmd(nc, in_maps, core_ids=list(range(NCORES)))
    out = np.concatenate([res.results[c]["out"] for c in range(NCORES)], axis=0)
    return out


# revision 5
# speedup vs baseline: 1.1729x; 1.1729x over previous
"""Trainium2 Bass kernel for AttributeAttentionModule.

y = attention over heads of QKV projections:
  Q = sa @ Wq.T + bq ; K = x @ Wk.T + bk ; V = x @ Wv.T + bv   (all [B, D])
  per-sample scores[h,g] = Q_h . K_g / 32 ; softmax over g ; out_h = sum_g w_hg V_g

Data-parallel over 8 NeuronCores (batch sharded). Matmuls run in float32r
(FP22) at 1 cycle/row. Weights are streamed once per group of 8 batch-tiles
(all 8 PSUM banks accumulate in parallel over the contraction dim). All HBM
operands are pre-tiled on the host so every DMA descriptor is a contiguous
12KB-per-partition block. Attention for group g overlaps group g+1 matmuls.
"""

import os
import sys

for _p in ("/opt/trn_rl_repo", "/root/.axon_site/_ro/trn_rl_repo"):
    if os.path.isdir(_p) and _p not in sys.path:
        sys.path.append(_p)

import numpy as np
from contextlib import ExitStack

B = 16384
D = 3072
H = 3
DH = D // H          # 1024
NCORES = 8
P = 128              # partition tile
NO = 512             # matmul moving free dim (one PSUM bank of fp32)
KGRP = 6             # k-tiles per weight DMA
KT = D // P          # 24 contraction tiles
NOT = D // NO        # 6 output-column tiles
NKG = KT // KGRP     # 4 weight DMAs per o-column

_CACHE = {}


def _build(bs=B // NCORES, gbt=8):
    """Build + compile the per-core program. bs = batch rows per core,
    gbt = batch tiles (of 128) per weight-streaming group."""
    import concourse.bass as bass
    import concourse.tile as tile
    from concourse import bacc, mybir

    f32 = mybir.dt.float32
    f32r = mybir.dt.float32r
    mult = mybir.AluOpType.mult
    add = mybir.AluOpType.add
    bypass = mybir.AluOpType.bypass
    Exp = mybir.ActivationFunctionType.Exp

    nbt = bs // P        # batch tiles per core
    ng = nbt // gbt      # weight-stream groups

    nc = bacc.Bacc(
        "TRN2", target_bir_lowering=False, debug=False, num_devices=NCORES
    )

    # pre-tiled inputs (see kernel() for host layouts)
    sa4 = nc.dram_tensor("sa4", [nbt, P, KT, P], f32r, kind="ExternalInput").ap()
    x4 = nc.dram_tensor("x4", [nbt, P, KT, P], f32r, kind="ExternalInput").ap()
    wT = {
        t: nc.dram_tensor(
            f"w{t}5", [NOT, NKG, P, KGRP, NO], f32r, kind="ExternalInput"
        ).ap()
        for t in "qkv"
    }
    biasd = {
        t: nc.dram_tensor(f"b{t}", [P, D], f32, kind="ExternalInput").ap()
        for t in "qkv"
    }
    outd = nc.dram_tensor("out", [bs, D], f32, kind="ExternalOutput").ap()

    with tile.TileContext(nc) as tc, ExitStack() as ctx:
        dram = ctx.enter_context(tc.tile_pool(name="dram", bufs=1, space="DRAM"))
        qkv_s = {t: dram.tile([bs, D], f32, tag=f"s{t}", name=f"s{t}") for t in "qkv"}

        apool = ctx.enter_context(tc.tile_pool(name="apool", bufs=1))
        wpool = ctx.enter_context(tc.tile_pool(name="wpool", bufs=2))
        bpool = ctx.enter_context(tc.tile_pool(name="bpool", bufs=1))
        ocpool = ctx.enter_context(tc.tile_pool(name="ocpool", bufs=3))
        pspool = ctx.enter_context(tc.tile_pool(name="psum", bufs=1, space="PSUM"))
        qkvp = ctx.enter_context(tc.tile_pool(name="qkvp", bufs=1))
        smallp = ctx.enter_context(tc.tile_pool(name="smallp", bufs=4))
        accp = ctx.enter_context(tc.tile_pool(name="accp", bufs=2))
        prodp = ctx.enter_context(tc.tile_pool(name="prodp", bufs=1))
        outp = ctx.enter_context(tc.tile_pool(name="outp", bufs=1))

        def load_act(src, g):
            tiles = []
            for i in range(gbt):
                t = apool.tile([P, KT, P], f32r, tag=f"a{i}", name=f"a{i}")
                nc.sync.dma_start(t[:], src[g * gbt + i])
                tiles.append(t)
            return tiles

        def proj(a_t, wTd, bias_d, dst, g):
            bias_t = bpool.tile([P, D], f32, tag="bias", name="bias")
            nc.sync.dma_start(bias_t[:], bias_d[:])
            for o in range(NOT):
                ps = [
                    pspool.tile([P, NO], f32, tag=f"ps{i}", name=f"ps{i}")
                    for i in range(gbt)
                ]
                for kg in range(NKG):
                    wt = wpool.tile([P, KGRP, NO], f32r, tag="w", name="w")
                    nc.sync.dma_start(wt[:], wTd[o, kg])
                    for j in range(KGRP):
                        k = kg * KGRP + j
                        for i in range(gbt):
                            nc.tensor.matmul(
                                ps[i][:],
                                a_t[i][:, k, :],
                                wt[:, j, :],
                                start=(k == 0),
                                stop=(k == KT - 1),
                            )
                for i in range(gbt):
                    oc = ocpool.tile([P, NO], f32, tag="oc", name="oc")
                    nc.vector.tensor_add(
                        oc[:], ps[i][:], bias_t[:, o * NO : (o + 1) * NO]
                    )
                    row0 = (g * gbt + i) * P
                    nc.sync.dma_start(
                        dst[row0 : row0 + P, o * NO : (o + 1) * NO], oc[:]
                    )

        def attn(bt):
            r0 = bt * P
            t3 = {}
            for t in "qkv":
                tt = qkvp.tile([P, D], f32, tag=t, name=f"t_{t}")
                nc.sync.dma_start(tt[:], qkv_s[t][r0 : r0 + P, :])
                t3[t] = tt
            s = smallp.tile([P, H * H], f32, tag="s", name="s")
            prod = prodp.tile([P, DH], f32, tag="prod", name="prod")
            for h in range(H):
                for g2 in range(H):
                    # fused row-wise dot: prod = Q_h * K_g ; s_hg = sum(prod)
                    nc.vector.scalar_tensor_tensor(
                        prod[:],
                        t3["q"][:, h * DH : (h + 1) * DH],
                        1.0,
                        t3["k"][:, g2 * DH : (g2 + 1) * DH],
                        op0=bypass,
                        op1=mult,
                        accum_out=s[:, h * H + g2 : h * H + g2 + 1],
                    )
            e = smallp.tile([P, H * H], f32, tag="e", name="e")
            nc.scalar.activation(e[:], s[:], Exp, scale=1.0 / 32.0)
            ssum = smallp.tile([P, H], f32, tag="ssum", name="ssum")
            nc.vector.tensor_reduce(
                ssum[:],
                e[:].rearrange("p (h g) -> p h g", h=H),
                axis=mybir.AxisListType.X,
                op=add,
            )
            rcp = smallp.tile([P, H], f32, tag="rcp", name="rcp")
            nc.vector.reciprocal(rcp[:], ssum[:])
            ot = outp.tile([P, D], f32, tag="o", name="o")
            for h in range(H):
                acc = accp.tile([P, DH], f32, tag="acc", name="acc")
                nc.vector.tensor_scalar_mul(
                    acc[:], t3["v"][:, 0:DH], e[:, h * H : h * H + 1]
                )
                for g2 in (1, 2):
                    nc.vector.scalar_tensor_tensor(
                        acc[:],
                        t3["v"][:, g2 * DH : (g2 + 1) * DH],
                        e[:, h * H + g2 : h * H + g2 + 1],
                        acc[:],
                        op0=mult,
                        op1=add,
                    )
                nc.vector.tensor_scalar_mul(
                    ot[:, h * DH : (h + 1) * DH], acc[:], rcp[:, h : h + 1]
                )
            nc.sync.dma_start(outd[r0 : r0 + P, :], ot[:])

        for g in range(ng):
            sa_t = load_act(sa4, g)
            proj(sa_t, wT["q"], biasd["q"], qkv_s["q"], g)
            x_t = load_act(x4, g)
            proj(x_t, wT["k"], biasd["k"], qkv_s["k"], g)
            proj(x_t, wT["v"], biasd["v"], qkv_s["v"], g)
            for i in range(gbt):
                attn(g * gbt + i)

    nc.compile()
    return nc


def _get_nc(bs=B // NCORES, gbt=8):
    key = (bs, gbt)
    if key not in _CACHE:
        _CACHE[key] = _build(bs, gbt)
    return _CACHE[key]


def _prep_weights(Wq, Wk, Wv, bq, bk, bv):
    """Pre-tile weights: w5[o, kg, p, j, n] = W.T[(kg*KGRP+j)*P + p, o*NO + n]."""
    ws = {}
    for nm, W in (("q", Wq), ("k", Wk), ("v", Wv)):
        wt = np.asarray(W, dtype=np.float32).T  # [in, out]
        w5 = wt.reshape(NKG, KGRP, P, NOT, NO).transpose(3, 0, 2, 1, 4)
        ws[nm] = np.ascontiguousarray(w5)
    bb = {
        nm: np.ascontiguousarray(
            np.broadcast_to(np.asarray(b, dtype=np.float32), (P, D))
        )
        for nm, b in (("q", bq), ("k", bk), ("v", bv))
    }
    return ws, bb


def _prep_act(a, bs):
    """Pre-tile activations per core: a4[bt, p, ko, b] = a[bt*P + b, ko*P + p]."""
    nbt = bs // P
    a4 = a.reshape(nbt, P, KT, P).transpose(0, 3, 2, 1)
    return np.ascontiguousarray(a4)


def _in_maps(x, sa, ws, bb, bs):
    maps = []
    for c in range(NCORES):
        r0 = c * bs
        maps.append(
            {
                "sa4": _prep_act(sa[r0 : r0 + bs], bs),
                "x4": _prep_act(x[r0 : r0 + bs], bs),
                "wq5": ws["q"],
                "wk5": ws["k"],
                "wv5": ws["v"],
                "bq": bb["q"],
                "bk": bb["k"],
                "bv": bb["v"],
            }
        )
    return maps


def kernel(x, synthetic_attributes, Wq, bq, Wk, bk, Wv, bv, **_ignored):
    from concourse import bass_utils

    x = np.asarray(x, dtype=np.float32)
    sa = np.asarray(synthetic_attributes, dtype=np.float32)
    bs = x.shape[0] // NCORES

    ws, bb = _prep_weights(Wq, Wk, Wv, bq, bk, bv)
    nc = _get_nc(bs=bs)
    in_maps = _in_maps(x, sa, ws, bb, bs)

    res = bass_utils.run_bass_kernel_spmd(nc, in_maps, core_ids=list(range(NCORES)))
    out = np.concatenate([res.results[c]["out"] for c in range(NCORES)], axis=0)
    return out


# revision 6
# speedup vs baseline: 1.1729x; 1.0000x over previous
"""Trainium2 Bass kernel for AttributeAttentionModule.

y = attention over heads of QKV projections:
  Q = sa @ Wq.T + bq ; K = x @ Wk.T + bk ; V = x @ Wv.T + bv   (all [B, D])
  per-sample scores[h,g] = Q_h . K_g / 32 ; softmax over g ; out_h = sum_g w_hg V_g

Data-parallel over 8 NeuronCores (batch sharded). Matmuls run in float32r
(FP22) at 1 cycle/row. Weights are streamed once per group of 8 batch-tiles
(all 8 PSUM banks accumulate in parallel over the contraction dim). All HBM
operands are pre-tiled on the host so every DMA descriptor is a contiguous
12KB-per-partition block. Attention is software-pipelined into the matmul
stream via filler chunks emitted after each o-sweep's PSUM copies.
"""

import os
import sys

for _p in ("/opt/trn_rl_repo", "/root/.axon_site/_ro/trn_rl_repo"):
    if os.path.isdir(_p) and _p not in sys.path:
        sys.path.append(_p)

import numpy as np
from contextlib import ExitStack

B = 16384
D = 3072
H = 3
DH = D // H          # 1024
NCORES = 8
P = 128              # partition tile
NO = 512             # matmul moving free dim (one PSUM bank of fp32)
KGRP = 6             # k-tiles per weight DMA
KT = D // P          # 24 contraction tiles
NOT = D // NO        # 6 output-column tiles
NKG = KT // KGRP     # 4 weight DMAs per o-column
KHALF = KT // 2      # stationary tiles arrive in two halves

_CACHE = {}


def _build(bs=B // NCORES, gbt=8):
    """Build + compile the per-core program. bs = batch rows per core,
    gbt = batch tiles (of 128) per weight-streaming group."""
    import concourse.bass as bass
    import concourse.tile as tile
    from concourse import bacc, mybir

    f32 = mybir.dt.float32
    f32r = mybir.dt.float32r
    mult = mybir.AluOpType.mult
    add = mybir.AluOpType.add
    bypass = mybir.AluOpType.bypass
    Exp = mybir.ActivationFunctionType.Exp

    nbt = bs // P        # batch tiles per core
    ng = nbt // gbt      # weight-stream groups

    nc = bacc.Bacc(
        "TRN2", target_bir_lowering=False, debug=False, num_devices=NCORES
    )

    # pre-tiled inputs (see kernel() for host layouts)
    sa4 = nc.dram_tensor("sa4", [nbt, P, KT, P], f32r, kind="ExternalInput").ap()
    x4 = nc.dram_tensor("x4", [nbt, P, KT, P], f32r, kind="ExternalInput").ap()
    wT = {
        t: nc.dram_tensor(
            f"w{t}5", [NOT, NKG, P, KGRP, NO], f32r, kind="ExternalInput"
        ).ap()
        for t in "qkv"
    }
    biasd = {
        t: nc.dram_tensor(f"b{t}", [P, D], f32, kind="ExternalInput").ap()
        for t in "qkv"
    }
    outd = nc.dram_tensor("out", [bs, D], f32, kind="ExternalOutput").ap()

    with tile.TileContext(nc) as tc, ExitStack() as ctx:
        dram = ctx.enter_context(tc.tile_pool(name="dram", bufs=1, space="DRAM"))
        qkv_s = {t: dram.tile([bs, D], f32, tag=f"s{t}", name=f"s{t}") for t in "qkv"}

        apool = ctx.enter_context(tc.tile_pool(name="apool", bufs=1))
        wpool = ctx.enter_context(tc.tile_pool(name="wpool", bufs=2))
        bpool = ctx.enter_context(tc.tile_pool(name="bpool", bufs=1))
        ocpool = ctx.enter_context(tc.tile_pool(name="ocpool", bufs=3))
        pspool = ctx.enter_context(tc.tile_pool(name="psum", bufs=1, space="PSUM"))
        qkvp = ctx.enter_context(tc.tile_pool(name="qkvp", bufs=1))
        smallp = ctx.enter_context(tc.tile_pool(name="smallp", bufs=4))
        accp = ctx.enter_context(tc.tile_pool(name="accp", bufs=2))
        prodp = ctx.enter_context(tc.tile_pool(name="prodp", bufs=1))
        outp = ctx.enter_context(tc.tile_pool(name="outp", bufs=1))

        pending = []  # attention chunk closures, drained between o-sweeps

        def filler():
            if pending:
                pending.pop(0)()

        def load_act(src, g):
            """Two half-k tiles per batch tile so matmuls can start on the
            first half while the second streams in."""
            tiles = []
            for i in range(gbt):
                lo = apool.tile([P, KHALF, P], f32r, tag=f"a{i}l", name=f"a{i}l")
                nc.sync.dma_start(lo[:], src[g * gbt + i, :, 0:KHALF, :])
                hi = apool.tile([P, KHALF, P], f32r, tag=f"a{i}h", name=f"a{i}h")
                nc.sync.dma_start(hi[:], src[g * gbt + i, :, KHALF:KT, :])
                tiles.append((lo, hi))
            return tiles

        def proj(items, wTd, bias_d, dst, first_o_fill=True):
            """items: list of (global_bt_index, (a_lo, a_hi))."""
            bias_t = bpool.tile([P, D], f32, tag="bias", name="bias")
            nc.sync.dma_start(bias_t[:], bias_d[:])
            for o in range(NOT):
                ps = {
                    bt: pspool.tile([P, NO], f32, tag=f"ps{j}", name=f"ps{j}")
                    for j, (bt, _) in enumerate(items)
                }
                for kg in range(NKG):
                    wt = wpool.tile([P, KGRP, NO], f32r, tag="w", name="w")
                    nc.sync.dma_start(wt[:], wTd[o, kg])
                    for j in range(KGRP):
                        k = kg * KGRP + j
                        for bt, (alo, ahi) in items:
                            a = alo if k < KHALF else ahi
                            nc.tensor.matmul(
                                ps[bt][:],
                                a[:, k % KHALF, :],
                                wt[:, j, :],
                                start=(k == 0),
                                stop=(k == KT - 1),
                            )
                for bt, _ in items:
                    oc = ocpool.tile([P, NO], f32, tag="oc", name="oc")
                    nc.vector.tensor_add(
                        oc[:], ps[bt][:], bias_t[:, o * NO : (o + 1) * NO]
                    )
                    nc.sync.dma_start(
                        dst[bt * P : bt * P + P, o * NO : (o + 1) * NO], oc[:]
                    )
                if first_o_fill or o > 0:
                    filler()

        def attn_chunks(bt):
            """Two closures per batch tile: A = load + scores + softmax,
            B = weighted V combine + store."""
            r0 = bt * P
            t3 = {}
            small = {}

            def chunk_a():
                for t in "qkv":
                    tt = qkvp.tile([P, D], f32, tag=t, name=f"t_{t}")
                    nc.sync.dma_start(tt[:], qkv_s[t][r0 : r0 + P, :])
                    t3[t] = tt
                s = smallp.tile([P, H * H], f32, tag="s", name="s")
                prod = prodp.tile([P, DH], f32, tag="prod", name="prod")
                for h in range(H):
                    for g2 in range(H):
                        # fused row-wise dot: prod = Q_h*K_g ; s_hg = sum(prod)
                        nc.vector.scalar_tensor_tensor(
                            prod[:],
                            t3["q"][:, h * DH : (h + 1) * DH],
                            1.0,
                            t3["k"][:, g2 * DH : (g2 + 1) * DH],
                            op0=bypass,
                            op1=mult,
                            accum_out=s[:, h * H + g2 : h * H + g2 + 1],
                        )
                e = smallp.tile([P, H * H], f32, tag="e", name="e")
                nc.scalar.activation(e[:], s[:], Exp, scale=1.0 / 32.0)
                ssum = smallp.tile([P, H], f32, tag="ssum", name="ssum")
                nc.vector.tensor_reduce(
                    ssum[:],
                    e[:].rearrange("p (h g) -> p h g", h=H),
                    axis=mybir.AxisListType.X,
                    op=add,
                )
                rcp = smallp.tile([P, H], f32, tag="rcp", name="rcp")
                nc.vector.reciprocal(rcp[:], ssum[:])
                small["e"] = e
                small["rcp"] = rcp

            def chunk_b():
                e, rcp = small["e"], small["rcp"]
                ot = outp.tile([P, D], f32, tag="o", name="o")
                for h in range(H):
                    acc = accp.tile([P, DH], f32, tag="acc", name="acc")
                    # first term on ScalarE (per-partition scalar scale)
                    nc.scalar.mul(acc[:], t3["v"][:, 0:DH], e[:, h * H : h * H + 1])
                    for g2 in (1, 2):
                        nc.vector.scalar_tensor_tensor(
                            acc[:],
                            t3["v"][:, g2 * DH : (g2 + 1) * DH],
                            e[:, h * H + g2 : h * H + g2 + 1],
                            acc[:],
                            op0=mult,
                            op1=add,
                        )
                    nc.scalar.mul(
                        ot[:, h * DH : (h + 1) * DH], acc[:], rcp[:, h : h + 1]
                    )
                nc.sync.dma_start(outd[r0 : r0 + P, :], ot[:])

            return [chunk_a, chunk_b]

        for g in range(ng):
            last = g == ng - 1
            bts = [g * gbt + i for i in range(gbt)]
            sa_t = load_act(sa4, g)
            proj(list(zip(bts, sa_t)), wT["q"], biasd["q"], qkv_s["q"])
            x_t = load_act(x4, g)
            proj(list(zip(bts, x_t)), wT["k"], biasd["k"], qkv_s["k"])
            items = list(zip(bts, x_t))
            if last and gbt >= 2:
                half = gbt // 2
                proj(items[:half], wT["v"], biasd["v"], qkv_s["v"])
                for bt in bts[:half]:
                    pending.extend(attn_chunks(bt))
                proj(items[half:], wT["v"], biasd["v"], qkv_s["v"])
                for bt in bts[half:]:
                    pending.extend(attn_chunks(bt))
            else:
                proj(items, wT["v"], biasd["v"], qkv_s["v"])
                for bt in bts:
                    pending.extend(attn_chunks(bt))
        while pending:
            pending.pop(0)()

    nc.compile()
    return nc


def _get_nc(bs=B // NCORES, gbt=8):
    key = (bs, gbt)
    if key not in _CACHE:
        _CACHE[key] = _build(bs, gbt)
    return _CACHE[key]


def _prep_weights(Wq, Wk, Wv, bq, bk, bv):
    """Pre-tile weights: w5[o, kg, p, j, n] = W.T[(kg*KGRP+j)*P + p, o*NO + n]."""
    ws = {}
    for nm, W in (("q", Wq), ("k", Wk), ("v", Wv)):
        wt = np.asarray(W, dtype=np.float32).T  # [in, out]
        w5 = wt.reshape(NKG, KGRP, P, NOT, NO).transpose(3, 0, 2, 1, 4)
        ws[nm] = np.ascontiguousarray(w5)
    bb = {
        nm: np.ascontiguousarray(
            np.broadcast_to(np.asarray(b, dtype=np.float32), (P, D))
        )
        for nm, b in (("q", bq), ("k", bk), ("v", bv))
    }
    return ws, bb


def _prep_act(a, bs):
    """Pre-tile activations per core: a4[bt, p, ko, b] = a[bt*P + b, ko*P + p]."""
    nbt = bs // P
    a4 = a.reshape(nbt, P, KT, P).transpose(0, 3, 2, 1)
    return np.ascontiguousarray(a4)


def _in_maps(x, sa, ws, bb, bs):
    maps = []
    for c in range(NCORES):
        r0 = c * bs
        maps.append(
            {
                "sa4": _prep_act(sa[r0 : r0 + bs], bs),
                "x4": _prep_act(x[r0 : r0 + bs], bs),
                "wq5": ws["q"],
                "wk5": ws["k"],
                "wv5": ws["v"],
                "bq": bb["q"],
                "bk": bb["k"],
                "bv": bb["v"],
            }
        )
    return maps


def kernel(x, synthetic_attributes, Wq, bq, Wk, bk, Wv, bv, **_ignored):
    from concourse import bass_utils

    x = np.asarray(x, dtype=np.float32)
    sa = np.asarray(synthetic_attributes, dtype=np.float32)
    bs = x.shape[0] // NCORES

    ws, bb = _prep_weights(Wq, Wk, Wv, bq, bk, bv)
    nc = _get_nc(bs=bs)
    in_maps = _in_maps(x, sa, ws, bb, bs)

    res = bass_utils.run_bass_kernel_spmd(nc, in_maps, core_ids=list(range(NCORES)))
    out = np.concatenate([res.results[c]["out"] for c in range(NCORES)], axis=0)
    return out
